# revision 63
# baseline (speedup 1.0000x reference)
"""Trainium2 Bass kernel for the MemoryReader (retrieval-knn) module.

Math (per batch b):
    a[m]     = sum_ck mk[ck, m]^2
    logits   = (2 * mk^T qk - a) / sqrt(CK)        # [THW, NQ]
    aff      = softmax(logits, axis=THW)
    out      = mv @ aff                            # [CV, NQ]

Shapes: B=4, CK=64, T=8, H=30, W=54 (THW=12960, NQ=1620), CV=512.
Sharding: 8 cores = (B=4) x (NQ halves of 810); softmax is over THW,
which every core owns fully, so no cross-core reduction is needed.

Score path (f32r, full PE rate): the squared-norm term is folded into
the score matmul by augmenting the contraction dim to K=128
(lhsT'=[mk;mk^2], rhs'=[qk;-0.5]); logits = 0.25*psum via ACT scale.

Readout path (fp8 DoubleRow, 2x PE rate, K=256 per matmul):
    ex  = 4*exp(logits)            (ACT bias=ln4; keeps all values well
                                    inside e4m3 range, max ~70 vs 240)
    e1  = fp8(ex)                  (GPSIMD copy, per tile)
    e2  = fp8(ex - e1)             (DVE sub, per tile; hi+lo
                                    reconstructs ex to ~0.2%)
    mv  = v1 + v2                  (host-packed fp8 hi+lo pair)
    acc = v1*e1 + v1*e2 + v2*e1    (3 DoubleRow matmuls per m-pair per
                                    cv chunk; v2*e2 ~ 1e-3^2, dropped)
    den = ones*(e1+e2)             (2 DoubleRow matmuls into a PSUM
                                    bank; every partition gets the full
                                    sum so DVE's reciprocal feeds the
                                    output muls directly)
The common factor 4 cancels in acc/den.  Operands are packed in m-PAIRS
of 128 rows: lhsT[p,t,c]=mv[256j+128t+p,c]; the e-tiles are stored
[p,n,t] and rearranged to [p,t,n]; mv rows are zero-padded to 13056 and
the last pair's t=1 exp tail is memset to 0 so garbage never enters
acc or den.  End-to-end rel err ~8e-3 (gate 2e-2).

Schedule: the exp->e1(Pool)->e2(DVE) chain takes ~3.4us per pair while
PE needs only ~1.9us, so the readout/den matmuls of pair j-1 are
emitted AFTER pair j+1's scores -- PE order [s(j+1), R(j-1)] gives the
quantize chain a two-pair budget, buffered in cheap SBUF e-tiles
(PSUM stays at 3 score bufs + 4 acc banks + 1 den bank).  Dummy PE
matmuls pre-ramp the p-state; output [128,4,QH] bf16 ships as two
[128,2,nq] DMAs per q-block; head DMAs ordered by first use.
"""

import math
import os
import sys

import ml_dtypes
import numpy as np

for _p in ("/opt/trn_rl_repo",):
    if _p not in sys.path and os.path.isdir(_p):
        sys.path.insert(0, _p)

B, CK, T, H, W = 4, 64, 8, 30, 54
CV = 512
THW = T * H * W          # 12960
NQ = H * W               # 1620
QH = NQ // 2             # 810   per-core query half
QBLKS = [(0, 512), (512, 298)]
P = 128
NPAIR = (THW + 255) // 256          # 51 pairs of 128-row tiles
THWP = NPAIR * 256                  # 13056 (mv zero-padded)
M_TILES = [(m0, min(P, THW - m0)) for m0 in range(0, THW, P)]  # 101x128 + 1x32
MKQ_CHUNK = 4 * P
PBLK = 2 * CV                       # fp8 elements per pair per mv tensor

_PROGRAM = None


def _build_program():
    import concourse.mybir as mybir
    import concourse.tile as tile
    from concourse import bacc

    f32 = mybir.dt.float32
    f32r = mybir.dt.float32r
    bf16 = mybir.dt.bfloat16
    fp8 = mybir.dt.float8e4
    Exp = mybir.ActivationFunctionType.Exp
    DR = mybir.MatmulPerfMode.DoubleRow

    nc = bacc.Bacc(
        "TRN2",
        target_bir_lowering=False,
        debug=False,
        enable_asserts=False,
        num_devices=8,
    )

    mkq = nc.dram_tensor("mkq", [P, THW], f32r, kind="ExternalInput").ap()
    qkc = nc.dram_tensor("qkc", [P, QH], f32r, kind="ExternalInput").ap()
    v1d = nc.dram_tensor("v1d", [P, NPAIR * PBLK], fp8, kind="ExternalInput").ap()
    v2d = nc.dram_tensor("v2d", [P, NPAIR * PBLK], fp8, kind="ExternalInput").ap()
    onesd = nc.dram_tensor("onesd", [P, 2, P], fp8, kind="ExternalInput").ap()
    out = nc.dram_tensor("out", [P, 4, QH], bf16, kind="ExternalOutput").ap()

    n_chunks = (THW + MKQ_CHUNK - 1) // MKQ_CHUNK

    with tile.TileContext(nc) as tc:
        with (
            tc.tile_pool(name="const", bufs=1) as cpool,
            tc.tile_pool(name="exf", bufs=4) as exfpool,
            tc.tile_pool(name="e8", bufs=4) as e8pool,
            tc.tile_pool(name="vec", bufs=2) as vpool,
            tc.tile_pool(name="outp", bufs=4) as opool,
            tc.tile_pool(name="score_ps", bufs=3, space="PSUM") as spspool,
            tc.tile_pool(name="acc_ps", bufs=1, space="PSUM") as apspool,
            tc.tile_pool(name="den_ps", bufs=1, space="PSUM") as dpspool,
        ):
            # Head DMAs (all SP; HWDGE is one serialized 625ns/DMA resource).
            qkc_sb = cpool.tile([P, QH], f32r, tag="qkc", name="qkc")
            nc.sync.dma_start(out=qkc_sb[:, : QBLKS[0][1]], in_=qkc[:, : QBLKS[0][1]])
            mkq_sb = cpool.tile([P, THW], f32r, tag="mkq", name="mkq")
            nc.sync.dma_start(out=mkq_sb[:, 0 : 3 * P], in_=mkq[:, 0 : 3 * P])
            ones_sb = cpool.tile([P, 2, P], fp8, tag="ones8", name="ones8")
            v1_sb = cpool.tile([P, NPAIR * PBLK], fp8, tag="v1", name="v1")
            v2_sb = cpool.tile([P, NPAIR * PBLK], fp8, tag="v2", name="v2")

            def mv_dma(j):
                nc.sync.dma_start(
                    out=v1_sb[:, j * PBLK : (j + 1) * PBLK],
                    in_=v1d[:, j * PBLK : (j + 1) * PBLK],
                )
                nc.sync.dma_start(
                    out=v2_sb[:, j * PBLK : (j + 1) * PBLK],
                    in_=v2d[:, j * PBLK : (j + 1) * PBLK],
                )

            nc.sync.dma_start(out=mkq_sb[:, 3 * P : 2 * MKQ_CHUNK], in_=mkq[:, 3 * P : 2 * MKQ_CHUNK])
            mv_dma(0)
            nc.sync.dma_start(out=ones_sb[:], in_=onesd[:])
            nc.sync.dma_start(out=qkc_sb[:, QBLKS[0][1] :], in_=qkc[:, QBLKS[0][1] :])
            next_chunk = 2
            for j in range(1, NPAIR):
                mv_dma(j)
                if j % 2 == 0 and next_chunk < n_chunks:
                    c0 = next_chunk * MKQ_CHUNK
                    c1 = min(c0 + MKQ_CHUNK, THW)
                    nc.sync.dma_start(out=mkq_sb[:, c0:c1], in_=mkq[:, c0:c1])
                    next_chunk += 1

            ones_mat = cpool.tile([P, P], f32, tag="ones_mat", name="ones_mat")
            nc.vector.memset(ones_mat[:], 1.0)
            ln4_sb = cpool.tile([P, 1], f32, tag="ln4", name="ln4")
            nc.vector.memset(ln4_sb[:], math.log(4.0))

            # PE p-state warmup (full clock needs ~3us continuous execution).
            warm = spspool.tile([P, QBLKS[0][1]], f32, tag="score", name="warm")
            for _ in range(7):
                nc.tensor.matmul(
                    warm[:, :P], lhsT=ones_mat[:], rhs=ones_mat[:], start=True, stop=True
                )

            def emit_scores(q0, nq, j):
                """Score matmuls for both tiles of pair j."""
                out_s = []
                for t in range(2):
                    k = 2 * j + t
                    if k >= len(M_TILES):
                        out_s.append(None)
                        continue
                    m0, mp = M_TILES[k]
                    s = spspool.tile([P, QBLKS[0][1]], f32, tag="score", name="score")
                    nc.tensor.matmul(
                        s[:mp, :nq],
                        lhsT=mkq_sb[:, m0 : m0 + mp],
                        rhs=qkc_sb[:, q0 : q0 + nq],
                        start=True,
                        stop=True,
                    )
                    out_s.append(s)
                return out_s

            QB0 = QBLKS[0][1]

            def emit_quant(nq, j, sa, sb):
                """exp + per-tile fp8 hi/lo quantize for pair j."""
                exf = exfpool.tile([P, QB0, 2], f32, tag="exf", name="exf")
                ma, mpa = M_TILES[2 * j]
                nc.scalar.activation(
                    exf[:mpa, :nq, 0], sa[:mpa, :nq], Exp, bias=ln4_sb[:mpa], scale=0.25
                )
                if 2 * j + 1 < len(M_TILES):
                    mb, mpb = M_TILES[2 * j + 1]
                    if mpb < P:
                        nc.vector.memset(exf[:, :nq, 1], 0.0)
                    nc.scalar.activation(
                        exf[:mpb, :nq, 1], sb[:mpb, :nq], Exp, bias=ln4_sb[:mpb], scale=0.25
                    )
                else:
                    nc.vector.memset(exf[:, :nq, 1], 0.0)
                e1 = e8pool.tile([P, QB0, 2], fp8, tag="e1", name="e1")
                e2 = e8pool.tile([P, QB0, 2], fp8, tag="e2", name="e2")
                for t in range(2):
                    nc.gpsimd.tensor_copy(e1[:, :nq, t], exf[:, :nq, t])
                    nc.vector.tensor_sub(e2[:, :nq, t], exf[:, :nq, t], e1[:, :nq, t])
                return e1, e2

            pre_scores = emit_scores(QBLKS[0][0], QBLKS[0][1], 0)
            pre_quant = None
            for qi, (q0, nq) in enumerate(QBLKS):
                accs = [apspool.tile([P, nq], f32, tag=f"acc{c}", name=f"acc{c}") for c in range(4)]
                den_ps = dpspool.tile([P, nq], f32, tag="den", name="den")
                recip_h = [None]

                def emit_reduce(j, e1, e2):
                    """den + readout DoubleRow matmuls for pair j."""
                    r1 = e1[:, :nq, :].rearrange("p n t -> p t n")
                    r2 = e2[:, :nq, :].rearrange("p n t -> p t n")
                    first, last = j == 0, j == NPAIR - 1
                    nc.tensor.matmul(
                        den_ps[:], lhsT=ones_sb[:], rhs=r1,
                        start=first, stop=False, perf_mode=DR,
                    )
                    nc.tensor.matmul(
                        den_ps[:], lhsT=ones_sb[:], rhs=r2,
                        start=False, stop=last, perf_mode=DR,
                    )
                    if last:
                        recip_h[0] = vpool.tile([P, nq], f32, tag="recip", name="recip")
                        nc.vector.reciprocal(recip_h[0][:], den_ps[:])
                    for c in range(4):
                        v1s = v1_sb[:, j * PBLK : (j + 1) * PBLK].rearrange(
                            "p (t c) -> p t c", t=2
                        )[:, :, c * P : (c + 1) * P]
                        v2s = v2_sb[:, j * PBLK : (j + 1) * PBLK].rearrange(
                            "p (t c) -> p t c", t=2
                        )[:, :, c * P : (c + 1) * P]
                        nc.tensor.matmul(
                            accs[c][:], lhsT=v1s, rhs=r1,
                            start=first, stop=False, perf_mode=DR,
                        )
                        nc.tensor.matmul(
                            accs[c][:], lhsT=v1s, rhs=r2,
                            start=False, stop=False, perf_mode=DR,
                        )
                        nc.tensor.matmul(
                            accs[c][:], lhsT=v2s, rhs=r1,
                            start=False, stop=last, perf_mode=DR,
                        )

                # Lag-1 pipeline: iteration j emits pair j+1's scores, pair
                # j's quantize chain, and pair j-1's readouts, so the
                # ACT->Pool->DVE chain has a two-pair window before PE needs
                # its result.
                pair_scores = pre_scores
                pending = None  # (j, e1, e2) awaiting readouts
                for j in range(NPAIR):
                    sa, sb = pair_scores
                    if j + 1 < NPAIR:
                        pair_scores = emit_scores(q0, nq, j + 1)
                    if j == 0 and pre_quant is not None:
                        e1, e2 = pre_quant
                    else:
                        e1, e2 = emit_quant(nq, j, sa, sb)
                    if pending is not None:
                        emit_reduce(*pending)
                    pending = (j, e1, e2)
                if qi + 1 < len(QBLKS):
                    # Pre-emit the next block's first-pair scores AND its
                    # quantize chain: PE chews the scores while the last
                    # pair drains, and crucially the next block's e2' lands
                    # on DVE ahead of this block's output muls.
                    nq0, nq1 = QBLKS[qi + 1]
                    pre_scores = emit_scores(nq0, nq1, 0)
                    pre_quant = emit_quant(nq1, 0, pre_scores[0], pre_scores[1])
                emit_reduce(*pending)

                for pair in range(2):
                    o2 = opool.tile([P, 2, nq], bf16, tag="out", name="out")
                    for jj in range(2):
                        c = 2 * pair + jj
                        nc.vector.tensor_mul(o2[:, jj, :], accs[c][:], recip_h[0][:])
                    nc.sync.dma_start(
                        out=out[:, 2 * pair : 2 * pair + 2, q0 : q0 + nq], in_=o2[:]
                    )

    nc.compile()
    return nc


def _get_program():
    global _PROGRAM
    if _PROGRAM is None:
        _PROGRAM = _build_program()
    return _PROGRAM


def _make_in_maps(mk, qk, mv):
    E4 = ml_dtypes.float8_e4m3
    mkf = np.ascontiguousarray(mk.reshape(B, CK, THW), dtype=np.float32)
    qkf = np.ascontiguousarray(qk.reshape(B, CK, NQ), dtype=np.float32)
    mvf = mv.reshape(B, CV, THW)
    onesd = np.ones((P, 2, P), dtype=E4)

    def pack(v):
        # [THWP, CV] -> [NPAIR, 2, 128, CV] -> [128, NPAIR, 2, CV] -> flat
        return np.ascontiguousarray(
            v.reshape(NPAIR, 2, P, CV).transpose(2, 0, 1, 3).reshape(P, NPAIR * PBLK)
        )

    in_maps = []
    for b in range(B):
        mkq_b = np.concatenate([mkf[b], mkf[b] * mkf[b]], axis=0)  # [128, THW]
        mvt = np.zeros((THWP, CV), dtype=np.float32)
        mvt[:THW] = mvf[b].T
        v1 = mvt.astype(E4)
        v2 = (mvt - v1.astype(np.float32)).astype(E4)
        v1p, v2p = pack(v1), pack(v2)
        for h in range(2):
            qkc_b = np.concatenate(
                [
                    qkf[b][:, h * QH : (h + 1) * QH],
                    np.full((CK, QH), -0.5, dtype=np.float32),
                ],
                axis=0,
            )
            in_maps.append(
                {
                    "mkq": mkq_b,
                    "qkc": np.ascontiguousarray(qkc_b),
                    "v1d": v1p,
                    "v2d": v2p,
                    "onesd": onesd,
                }
            )
    return in_maps


def kernel(mk, qk, mv, _trace=False, _results_out=None):
    from concourse import bass_utils

    nc = _get_program()
    in_maps = _make_in_maps(np.asarray(mk), np.asarray(qk), np.asarray(mv))
    res = bass_utils.run_bass_kernel_spmd(
        nc, in_maps, core_ids=list(range(8)), trace=_trace
    )
    if _results_out is not None:
        _results_out.append(res)

    full = np.empty((B, CV, NQ), dtype=np.float32)
    for b in range(B):
        for h in range(2):
            o = res.results[2 * b + h]["out"].astype(np.float32)  # [128, 4, QH]
            full[b][:, h * QH : (h + 1) * QH] = o.transpose(1, 0, 2).reshape(CV, QH)
    return full.reshape(B, CV, H, W)


# revision 64
# speedup vs baseline: 1.0462x; 1.0462x over previous
"""Trainium2 Bass kernel for the MemoryReader (retrieval-knn) module.

Math (per batch b):
    a[m]     = sum_ck mk[ck, m]^2
    logits   = (2 * mk^T qk - a) / sqrt(CK)        # [THW, NQ]
    aff      = softmax(logits, axis=THW)
    out      = mv @ aff                            # [CV, NQ]

Shapes: B=4, CK=64, T=8, H=30, W=54 (THW=12960, NQ=1620), CV=512.
Sharding: 8 cores = (B=4) x (NQ halves of 810); softmax is over THW,
which every core owns fully, so no cross-core reduction is needed.

Score path (f32r, full PE rate): the squared-norm term is folded into
the score matmul by augmenting the contraction dim to K=128
(lhsT'=[mk;mk^2], rhs'=[qk;-0.5]); logits = 0.25*psum via ACT scale.

Readout path (fp8 DoubleRow, 2x PE rate, K=256 per matmul):
    ex  = 4*exp(logits)            (ACT bias=ln4; keeps all values well
                                    inside e4m3 range, max ~70 vs 240)
    e1  = fp8(ex)                  (GPSIMD copy, per tile)
    e2  = fp8(ex - e1)             (DVE sub, per tile; hi+lo
                                    reconstructs ex to ~0.2%)
    mv  = v1 + v2                  (host-packed fp8 hi+lo pair)
    acc = v1*e1 + v1*e2 + v2*e1    (3 DoubleRow matmuls per m-pair per
                                    cv chunk; v2*e2 ~ 1e-3^2, dropped)
    den = ones*(e1+e2)             (2 DoubleRow matmuls into a PSUM
                                    bank; every partition gets the full
                                    sum so DVE's reciprocal feeds the
                                    output muls directly)
The common factor 4 cancels in acc/den.  Operands are packed in m-PAIRS
of 128 rows: lhsT[p,t,c]=mv[256j+128t+p,c]; the e-tiles are stored
[p,n,t] and rearranged to [p,t,n]; mv rows are zero-padded to 13056 and
the last pair's t=1 exp tail is memset to 0 so garbage never enters
acc or den.  End-to-end rel err ~8e-3 (gate 2e-2).

Schedule: the exp->e1(Pool)->e2(DVE) chain takes ~3.4us per pair while
PE needs only ~1.9us, so the readout/den matmuls of pair j-1 are
emitted AFTER pair j+1's scores -- PE order [s(j+1), R(j-1)] gives the
quantize chain a two-pair budget, buffered in cheap SBUF e-tiles
(PSUM stays at 3 score bufs + 4 acc banks + 1 den bank).  Dummy PE
matmuls pre-ramp the p-state; output [128,4,QH] bf16 ships as two
[128,2,nq] DMAs per q-block; head DMAs ordered by first use.
"""

import math
import os
import sys

import ml_dtypes
import numpy as np

for _p in ("/opt/trn_rl_repo",):
    if _p not in sys.path and os.path.isdir(_p):
        sys.path.insert(0, _p)

B, CK, T, H, W = 4, 64, 8, 30, 54
CV = 512
THW = T * H * W          # 12960
NQ = H * W               # 1620
QH = NQ // 2             # 810   per-core query half
QBLKS = [(0, 512), (512, 298)]
P = 128
NPAIR = (THW + 255) // 256          # 51 pairs of 128-row tiles
THWP = NPAIR * 256                  # 13056 (mv zero-padded)
M_TILES = [(m0, min(P, THW - m0)) for m0 in range(0, THW, P)]  # 101x128 + 1x32
MKQ_CHUNK = 4 * P
PBLK = 2 * CV                       # fp8 elements per pair per mv tensor

_PROGRAM = None


def _build_program():
    import concourse.mybir as mybir
    import concourse.tile as tile
    from concourse import bacc

    f32 = mybir.dt.float32
    f32r = mybir.dt.float32r
    bf16 = mybir.dt.bfloat16
    fp8 = mybir.dt.float8e4
    Exp = mybir.ActivationFunctionType.Exp
    DR = mybir.MatmulPerfMode.DoubleRow

    nc = bacc.Bacc(
        "TRN2",
        target_bir_lowering=False,
        debug=False,
        enable_asserts=False,
        num_devices=8,
    )

    mkq = nc.dram_tensor("mkq", [P, THW], f32r, kind="ExternalInput").ap()
    qkc = nc.dram_tensor("qkc", [P, QH], f32r, kind="ExternalInput").ap()
    v1d = nc.dram_tensor("v1d", [P, NPAIR * PBLK], fp8, kind="ExternalInput").ap()
    v2d = nc.dram_tensor("v2d", [P, NPAIR * PBLK], fp8, kind="ExternalInput").ap()
    onesd = nc.dram_tensor("onesd", [P, 2, P], fp8, kind="ExternalInput").ap()
    out = nc.dram_tensor("out", [P, 4, QH], bf16, kind="ExternalOutput").ap()

    n_chunks = (THW + MKQ_CHUNK - 1) // MKQ_CHUNK

    with tile.TileContext(nc) as tc:
        with (
            tc.tile_pool(name="const", bufs=1) as cpool,
            tc.tile_pool(name="exf", bufs=4) as exfpool,
            tc.tile_pool(name="e8", bufs=4) as e8pool,
            tc.tile_pool(name="vec", bufs=2) as vpool,
            tc.tile_pool(name="outp", bufs=4) as opool,
            tc.tile_pool(name="score_ps", bufs=3, space="PSUM") as spspool,
            tc.tile_pool(name="acc_ps", bufs=1, space="PSUM") as apspool,
            tc.tile_pool(name="den_ps", bufs=1, space="PSUM") as dpspool,
        ):
            # Head DMAs (all SP; HWDGE is one serialized 625ns/DMA resource).
            qkc_sb = cpool.tile([P, QH], f32r, tag="qkc", name="qkc")
            nc.sync.dma_start(out=qkc_sb[:, : QBLKS[0][1]], in_=qkc[:, : QBLKS[0][1]])
            mkq_sb = cpool.tile([P, THW], f32r, tag="mkq", name="mkq")
            nc.sync.dma_start(out=mkq_sb[:, 0 : 3 * P], in_=mkq[:, 0 : 3 * P])
            ones_sb = cpool.tile([P, 2, P], fp8, tag="ones8", name="ones8")
            v1_sb = cpool.tile([P, NPAIR * PBLK], fp8, tag="v1", name="v1")
            v2_sb = cpool.tile([P, NPAIR * PBLK], fp8, tag="v2", name="v2")

            def mv_dma(j):
                nc.sync.dma_start(
                    out=v1_sb[:, j * PBLK : (j + 1) * PBLK],
                    in_=v1d[:, j * PBLK : (j + 1) * PBLK],
                )
                nc.sync.dma_start(
                    out=v2_sb[:, j * PBLK : (j + 1) * PBLK],
                    in_=v2d[:, j * PBLK : (j + 1) * PBLK],
                )

            nc.sync.dma_start(out=mkq_sb[:, 3 * P : 2 * MKQ_CHUNK], in_=mkq[:, 3 * P : 2 * MKQ_CHUNK])
            mv_dma(0)
            nc.sync.dma_start(out=ones_sb[:], in_=onesd[:])
            nc.sync.dma_start(out=qkc_sb[:, QBLKS[0][1] :], in_=qkc[:, QBLKS[0][1] :])
            next_chunk = 2
            for j in range(1, NPAIR):
                mv_dma(j)
                if j % 2 == 0 and next_chunk < n_chunks:
                    c0 = next_chunk * MKQ_CHUNK
                    c1 = min(c0 + MKQ_CHUNK, THW)
                    nc.sync.dma_start(out=mkq_sb[:, c0:c1], in_=mkq[:, c0:c1])
                    next_chunk += 1

            ones_mat = cpool.tile([P, P], f32, tag="ones_mat", name="ones_mat")
            nc.vector.memset(ones_mat[:], 1.0)
            ln4_sb = cpool.tile([P, 1], f32, tag="ln4", name="ln4")
            nc.vector.memset(ln4_sb[:], math.log(4.0))

            # PE p-state warmup (full clock needs ~3us continuous execution).
            warm = spspool.tile([P, QBLKS[0][1]], f32, tag="score", name="warm")
            for _ in range(7):
                nc.tensor.matmul(
                    warm[:, :P], lhsT=ones_mat[:], rhs=ones_mat[:], start=True, stop=True
                )

            def emit_scores(q0, nq, j):
                """Score matmuls for both tiles of pair j."""
                out_s = []
                for t in range(2):
                    k = 2 * j + t
                    if k >= len(M_TILES):
                        out_s.append(None)
                        continue
                    m0, mp = M_TILES[k]
                    s = spspool.tile([P, QBLKS[0][1]], f32, tag="score", name="score")
                    nc.tensor.matmul(
                        s[:mp, :nq],
                        lhsT=mkq_sb[:, m0 : m0 + mp],
                        rhs=qkc_sb[:, q0 : q0 + nq],
                        start=True,
                        stop=True,
                    )
                    out_s.append(s)
                return out_s

            QB0 = QBLKS[0][1]

            def emit_quant(nq, j, sa, sb):
                """exp + per-tile fp8 hi/lo quantize for pair j."""
                exf = exfpool.tile([P, QB0, 2], f32, tag="exf", name="exf")
                ma, mpa = M_TILES[2 * j]
                nc.scalar.activation(
                    exf[:mpa, :nq, 0], sa[:mpa, :nq], Exp, bias=ln4_sb[:mpa], scale=0.25
                )
                if 2 * j + 1 < len(M_TILES):
                    mb, mpb = M_TILES[2 * j + 1]
                    if mpb < P:
                        nc.vector.memset(exf[:, :nq, 1], 0.0)
                    nc.scalar.activation(
                        exf[:mpb, :nq, 1], sb[:mpb, :nq], Exp, bias=ln4_sb[:mpb], scale=0.25
                    )
                else:
                    nc.vector.memset(exf[:, :nq, 1], 0.0)
                e1 = e8pool.tile([P, QB0, 2], fp8, tag="e1", name="e1")
                e2 = e8pool.tile([P, QB0, 2], fp8, tag="e2", name="e2")
                for t in range(2):
                    nc.gpsimd.tensor_copy(e1[:, :nq, t], exf[:, :nq, t])
                    nc.vector.tensor_sub(e2[:, :nq, t], exf[:, :nq, t], e1[:, :nq, t])
                return e1, e2

            pre_scores = emit_scores(QBLKS[0][0], QBLKS[0][1], 0)
            pre_quant = None
            for qi, (q0, nq) in enumerate(QBLKS):
                accs = [apspool.tile([P, nq], f32, tag=f"acc{c}", name=f"acc{c}") for c in range(4)]
                den_ps = dpspool.tile([P, nq], f32, tag="den", name="den")
                recip_h = [None]

                def emit_reduce(j, e1, e2):
                    """den + readout DoubleRow matmuls for pair j."""
                    r1 = e1[:, :nq, :].rearrange("p n t -> p t n")
                    r2 = e2[:, :nq, :].rearrange("p n t -> p t n")
                    first, last = j == 0, j == NPAIR - 1
                    # den from e1 only: e2 is a round-to-nearest residual
                    # whose sum cancels (measured +5e-4 rel err); saves one
                    # DoubleRow matmul per pair.
                    nc.tensor.matmul(
                        den_ps[:], lhsT=ones_sb[:], rhs=r1,
                        start=first, stop=last, perf_mode=DR,
                    )
                    if last:
                        recip_h[0] = vpool.tile([P, nq], f32, tag="recip", name="recip")
                        nc.vector.reciprocal(recip_h[0][:], den_ps[:])
                    for c in range(4):
                        v1s = v1_sb[:, j * PBLK : (j + 1) * PBLK].rearrange(
                            "p (t c) -> p t c", t=2
                        )[:, :, c * P : (c + 1) * P]
                        v2s = v2_sb[:, j * PBLK : (j + 1) * PBLK].rearrange(
                            "p (t c) -> p t c", t=2
                        )[:, :, c * P : (c + 1) * P]
                        nc.tensor.matmul(
                            accs[c][:], lhsT=v1s, rhs=r1,
                            start=first, stop=False, perf_mode=DR,
                        )
                        nc.tensor.matmul(
                            accs[c][:], lhsT=v1s, rhs=r2,
                            start=False, stop=False, perf_mode=DR,
                        )
                        nc.tensor.matmul(
                            accs[c][:], lhsT=v2s, rhs=r1,
                            start=False, stop=last, perf_mode=DR,
                        )

                # Lag-1 pipeline: iteration j emits pair j+1's scores, pair
                # j's quantize chain, and pair j-1's readouts, so the
                # ACT->Pool->DVE chain has a two-pair window before PE needs
                # its result.
                pair_scores = pre_scores
                pending = None  # (j, e1, e2) awaiting readouts
                for j in range(NPAIR):
                    sa, sb = pair_scores
                    if j + 1 < NPAIR:
                        pair_scores = emit_scores(q0, nq, j + 1)
                    if j == 0 and pre_quant is not None:
                        e1, e2 = pre_quant
                    else:
                        e1, e2 = emit_quant(nq, j, sa, sb)
                    if pending is not None:
                        emit_reduce(*pending)
                    pending = (j, e1, e2)
                if qi + 1 < len(QBLKS):
                    # Pre-emit the next block's first-pair scores AND its
                    # quantize chain: PE chews the scores while the last
                    # pair drains, and crucially the next block's e2' lands
                    # on DVE ahead of this block's output muls.
                    nq0, nq1 = QBLKS[qi + 1]
                    pre_scores = emit_scores(nq0, nq1, 0)
                    pre_quant = emit_quant(nq1, 0, pre_scores[0], pre_scores[1])
                emit_reduce(*pending)

                for pair in range(2):
                    o2 = opool.tile([P, 2, nq], bf16, tag="out", name="out")
                    for jj in range(2):
                        c = 2 * pair + jj
                        nc.vector.tensor_mul(o2[:, jj, :], accs[c][:], recip_h[0][:])
                    nc.sync.dma_start(
                        out=out[:, 2 * pair : 2 * pair + 2, q0 : q0 + nq], in_=o2[:]
                    )

    nc.compile()
    return nc


def _get_program():
    global _PROGRAM
    if _PROGRAM is None:
        _PROGRAM = _build_program()
    return _PROGRAM


def _make_in_maps(mk, qk, mv):
    E4 = ml_dtypes.float8_e4m3
    mkf = np.ascontiguousarray(mk.reshape(B, CK, THW), dtype=np.float32)
    qkf = np.ascontiguousarray(qk.reshape(B, CK, NQ), dtype=np.float32)
    mvf = mv.reshape(B, CV, THW)
    onesd = np.ones((P, 2, P), dtype=E4)

    def pack(v):
        # [THWP, CV] -> [NPAIR, 2, 128, CV] -> [128, NPAIR, 2, CV] -> flat
        return np.ascontiguousarray(
            v.reshape(NPAIR, 2, P, CV).transpose(2, 0, 1, 3).reshape(P, NPAIR * PBLK)
        )

    in_maps = []
    for b in range(B):
        mkq_b = np.concatenate([mkf[b], mkf[b] * mkf[b]], axis=0)  # [128, THW]
        mvt = np.zeros((THWP, CV), dtype=np.float32)
        mvt[:THW] = mvf[b].T
        v1 = mvt.astype(E4)
        v2 = (mvt - v1.astype(np.float32)).astype(E4)
        v1p, v2p = pack(v1), pack(v2)
        for h in range(2):
            qkc_b = np.concatenate(
                [
                    qkf[b][:, h * QH : (h + 1) * QH],
                    np.full((CK, QH), -0.5, dtype=np.float32),
                ],
                axis=0,
            )
            in_maps.append(
                {
                    "mkq": mkq_b,
                    "qkc": np.ascontiguousarray(qkc_b),
                    "v1d": v1p,
                    "v2d": v2p,
                    "onesd": onesd,
                }
            )
    return in_maps


def kernel(mk, qk, mv, _trace=False, _results_out=None):
    from concourse import bass_utils

    nc = _get_program()
    in_maps = _make_in_maps(np.asarray(mk), np.asarray(qk), np.asarray(mv))
    res = bass_utils.run_bass_kernel_spmd(
        nc, in_maps, core_ids=list(range(8)), trace=_trace
    )
    if _results_out is not None:
        _results_out.append(res)

    full = np.empty((B, CV, NQ), dtype=np.float32)
    for b in range(B):
        for h in range(2):
            o = res.results[2 * b + h]["out"].astype(np.float32)  # [128, 4, QH]
            full[b][:, h * QH : (h + 1) * QH] = o.transpose(1, 0, 2).reshape(CV, QH)
    return full.reshape(B, CV, H, W)


# revision 65
# speedup vs baseline: 1.0466x; 1.0004x over previous
"""Trainium2 Bass kernel for the MemoryReader (retrieval-knn) module.

Math (per batch b):
    a[m]     = sum_ck mk[ck, m]^2
    logits   = (2 * mk^T qk - a) / sqrt(CK)        # [THW, NQ]
    aff      = softmax(logits, axis=THW)
    out      = mv @ aff                            # [CV, NQ]

Shapes: B=4, CK=64, T=8, H=30, W=54 (THW=12960, NQ=1620), CV=512.
Sharding: 8 cores = (B=4) x (NQ halves of 810); softmax is over THW,
which every core owns fully, so no cross-core reduction is needed.

Score path (f32r, full PE rate): the squared-norm term is folded into
the score matmul by augmenting the contraction dim to K=128
(lhsT'=[mk;mk^2], rhs'=[qk;-0.5]); logits = 0.25*psum via ACT scale.

Readout path (fp8 DoubleRow, 2x PE rate, K=256 per matmul):
    ex  = 4*exp(logits)            (ACT bias=ln4; keeps all values well
                                    inside e4m3 range, max ~70 vs 240)
    e1  = fp8(ex)                  (GPSIMD copy, per tile)
    e2  = fp8(ex - e1)             (DVE sub, per tile; hi+lo
                                    reconstructs ex to ~0.2%)
    mv  = v1 + v2                  (host-packed fp8 hi+lo pair)
    acc = v1*e1 + v1*e2 + v2*e1    (3 DoubleRow matmuls per m-pair per
                                    cv chunk; v2*e2 ~ 1e-3^2, dropped)
    den = ones*(e1+e2)             (2 DoubleRow matmuls into a PSUM
                                    bank; every partition gets the full
                                    sum so DVE's reciprocal feeds the
                                    output muls directly)
The common factor 4 cancels in acc/den.  Operands are packed in m-PAIRS
of 128 rows: lhsT[p,t,c]=mv[256j+128t+p,c]; the e-tiles are stored
[p,n,t] and rearranged to [p,t,n]; mv rows are zero-padded to 13056 and
the last pair's t=1 exp tail is memset to 0 so garbage never enters
acc or den.  End-to-end rel err ~8e-3 (gate 2e-2).

Schedule: the exp->e1(Pool)->e2(DVE) chain takes ~3.4us per pair while
PE needs only ~1.9us, so the readout/den matmuls of pair j-1 are
emitted AFTER pair j+1's scores -- PE order [s(j+1), R(j-1)] gives the
quantize chain a two-pair budget, buffered in cheap SBUF e-tiles
(PSUM stays at 3 score bufs + 4 acc banks + 1 den bank).  Dummy PE
matmuls pre-ramp the p-state; output [128,4,QH] bf16 ships as two
[128,2,nq] DMAs per q-block; head DMAs ordered by first use.
"""

import math
import os
import sys

import ml_dtypes
import numpy as np

for _p in ("/opt/trn_rl_repo",):
    if _p not in sys.path and os.path.isdir(_p):
        sys.path.insert(0, _p)

B, CK, T, H, W = 4, 64, 8, 30, 54
CV = 512
THW = T * H * W          # 12960
NQ = H * W               # 1620
QH = NQ // 2             # 810   per-core query half
QBLKS = [(0, 512), (512, 298)]
P = 128
NPAIR = (THW + 255) // 256          # 51 pairs of 128-row tiles
THWP = NPAIR * 256                  # 13056 (mv zero-padded)
M_TILES = [(m0, min(P, THW - m0)) for m0 in range(0, THW, P)]  # 101x128 + 1x32
MKQ_CHUNK = 4 * P
PBLK = 2 * CV                       # fp8 elements per pair per mv tensor

_PROGRAM = None


def _build_program():
    import concourse.mybir as mybir
    import concourse.tile as tile
    from concourse import bacc

    f32 = mybir.dt.float32
    f32r = mybir.dt.float32r
    bf16 = mybir.dt.bfloat16
    fp8 = mybir.dt.float8e4
    Exp = mybir.ActivationFunctionType.Exp
    DR = mybir.MatmulPerfMode.DoubleRow

    nc = bacc.Bacc(
        "TRN2",
        target_bir_lowering=False,
        debug=False,
        enable_asserts=False,
        num_devices=8,
    )

    mkq = nc.dram_tensor("mkq", [P, THW], f32r, kind="ExternalInput").ap()
    qkc = nc.dram_tensor("qkc", [P, QH], f32r, kind="ExternalInput").ap()
    v1d = nc.dram_tensor("v1d", [P, NPAIR * PBLK], fp8, kind="ExternalInput").ap()
    v2d = nc.dram_tensor("v2d", [P, NPAIR * PBLK], fp8, kind="ExternalInput").ap()
    onesd = nc.dram_tensor("onesd", [P, 2, P], fp8, kind="ExternalInput").ap()
    out = nc.dram_tensor("out", [P, 4, QH], bf16, kind="ExternalOutput").ap()

    n_chunks = (THW + MKQ_CHUNK - 1) // MKQ_CHUNK

    with tile.TileContext(nc) as tc:
        with (
            tc.tile_pool(name="const", bufs=1) as cpool,
            tc.tile_pool(name="exf", bufs=5) as exfpool,
            tc.tile_pool(name="e8", bufs=5) as e8pool,
            tc.tile_pool(name="vec", bufs=2) as vpool,
            tc.tile_pool(name="outp", bufs=4) as opool,
            tc.tile_pool(name="score_ps", bufs=3, space="PSUM") as spspool,
            tc.tile_pool(name="acc_ps", bufs=1, space="PSUM") as apspool,
            tc.tile_pool(name="den_ps", bufs=1, space="PSUM") as dpspool,
        ):
            # Head DMAs (all SP; HWDGE is one serialized 625ns/DMA resource).
            qkc_sb = cpool.tile([P, QH], f32r, tag="qkc", name="qkc")
            nc.sync.dma_start(out=qkc_sb[:, : QBLKS[0][1]], in_=qkc[:, : QBLKS[0][1]])
            mkq_sb = cpool.tile([P, THW], f32r, tag="mkq", name="mkq")
            nc.sync.dma_start(out=mkq_sb[:, 0 : 3 * P], in_=mkq[:, 0 : 3 * P])
            ones_sb = cpool.tile([P, 2, P], fp8, tag="ones8", name="ones8")
            v1_sb = cpool.tile([P, NPAIR * PBLK], fp8, tag="v1", name="v1")
            v2_sb = cpool.tile([P, NPAIR * PBLK], fp8, tag="v2", name="v2")

            def mv_dma(j):
                nc.sync.dma_start(
                    out=v1_sb[:, j * PBLK : (j + 1) * PBLK],
                    in_=v1d[:, j * PBLK : (j + 1) * PBLK],
                )
                nc.sync.dma_start(
                    out=v2_sb[:, j * PBLK : (j + 1) * PBLK],
                    in_=v2d[:, j * PBLK : (j + 1) * PBLK],
                )

            nc.sync.dma_start(out=mkq_sb[:, 3 * P : 2 * MKQ_CHUNK], in_=mkq[:, 3 * P : 2 * MKQ_CHUNK])
            mv_dma(0)
            nc.sync.dma_start(out=ones_sb[:], in_=onesd[:])
            nc.sync.dma_start(out=qkc_sb[:, QBLKS[0][1] :], in_=qkc[:, QBLKS[0][1] :])
            next_chunk = 2
            for j in range(1, NPAIR):
                mv_dma(j)
                if j % 2 == 0 and next_chunk < n_chunks:
                    c0 = next_chunk * MKQ_CHUNK
                    c1 = min(c0 + MKQ_CHUNK, THW)
                    nc.sync.dma_start(out=mkq_sb[:, c0:c1], in_=mkq[:, c0:c1])
                    next_chunk += 1

            ones_mat = cpool.tile([P, P], f32, tag="ones_mat", name="ones_mat")
            nc.vector.memset(ones_mat[:], 1.0)
            ln4_sb = cpool.tile([P, 1], f32, tag="ln4", name="ln4")
            nc.vector.memset(ln4_sb[:], math.log(4.0))

            # PE p-state warmup (full clock needs ~3us continuous execution).
            warm = spspool.tile([P, QBLKS[0][1]], f32, tag="score", name="warm")
            for _ in range(7):
                nc.tensor.matmul(
                    warm[:, :P], lhsT=ones_mat[:], rhs=ones_mat[:], start=True, stop=True
                )

            def emit_scores(q0, nq, j):
                """Score matmuls for both tiles of pair j."""
                out_s = []
                for t in range(2):
                    k = 2 * j + t
                    if k >= len(M_TILES):
                        out_s.append(None)
                        continue
                    m0, mp = M_TILES[k]
                    s = spspool.tile([P, QBLKS[0][1]], f32, tag="score", name="score")
                    nc.tensor.matmul(
                        s[:mp, :nq],
                        lhsT=mkq_sb[:, m0 : m0 + mp],
                        rhs=qkc_sb[:, q0 : q0 + nq],
                        start=True,
                        stop=True,
                    )
                    out_s.append(s)
                return out_s

            QB0 = QBLKS[0][1]

            def emit_quant(nq, j, sa, sb):
                """exp + per-tile fp8 hi/lo quantize for pair j."""
                exf = exfpool.tile([P, QB0, 2], f32, tag="exf", name="exf")
                ma, mpa = M_TILES[2 * j]
                nc.scalar.activation(
                    exf[:mpa, :nq, 0], sa[:mpa, :nq], Exp, bias=ln4_sb[:mpa], scale=0.25
                )
                if 2 * j + 1 < len(M_TILES):
                    mb, mpb = M_TILES[2 * j + 1]
                    if mpb < P:
                        nc.vector.memset(exf[:, :nq, 1], 0.0)
                    nc.scalar.activation(
                        exf[:mpb, :nq, 1], sb[:mpb, :nq], Exp, bias=ln4_sb[:mpb], scale=0.25
                    )
                else:
                    nc.vector.memset(exf[:, :nq, 1], 0.0)
                e1 = e8pool.tile([P, QB0, 2], fp8, tag="e1", name="e1")
                e2 = e8pool.tile([P, QB0, 2], fp8, tag="e2", name="e2")
                for t in range(2):
                    nc.gpsimd.tensor_copy(e1[:, :nq, t], exf[:, :nq, t])
                    nc.vector.tensor_sub(e2[:, :nq, t], exf[:, :nq, t], e1[:, :nq, t])
                return e1, e2

            pre_scores = emit_scores(QBLKS[0][0], QBLKS[0][1], 0)
            pre_quant = None
            for qi, (q0, nq) in enumerate(QBLKS):
                accs = [apspool.tile([P, nq], f32, tag=f"acc{c}", name=f"acc{c}") for c in range(4)]
                den_ps = dpspool.tile([P, nq], f32, tag="den", name="den")
                recip_h = [None]

                def emit_reduce(j, e1, e2):
                    """den + readout DoubleRow matmuls for pair j."""
                    r1 = e1[:, :nq, :].rearrange("p n t -> p t n")
                    r2 = e2[:, :nq, :].rearrange("p n t -> p t n")
                    first, last = j == 0, j == NPAIR - 1
                    # den from e1 only: e2 is a round-to-nearest residual
                    # whose sum cancels (measured +5e-4 rel err); saves one
                    # DoubleRow matmul per pair.
                    nc.tensor.matmul(
                        den_ps[:], lhsT=ones_sb[:], rhs=r1,
                        start=first, stop=last, perf_mode=DR,
                    )
                    if last:
                        recip_h[0] = vpool.tile([P, nq], f32, tag="recip", name="recip")
                        nc.vector.reciprocal(recip_h[0][:], den_ps[:])
                    for c in range(4):
                        v1s = v1_sb[:, j * PBLK : (j + 1) * PBLK].rearrange(
                            "p (t c) -> p t c", t=2
                        )[:, :, c * P : (c + 1) * P]
                        v2s = v2_sb[:, j * PBLK : (j + 1) * PBLK].rearrange(
                            "p (t c) -> p t c", t=2
                        )[:, :, c * P : (c + 1) * P]
                        nc.tensor.matmul(
                            accs[c][:], lhsT=v1s, rhs=r1,
                            start=first, stop=False, perf_mode=DR,
                        )
                        nc.tensor.matmul(
                            accs[c][:], lhsT=v1s, rhs=r2,
                            start=False, stop=False, perf_mode=DR,
                        )
                        nc.tensor.matmul(
                            accs[c][:], lhsT=v2s, rhs=r1,
                            start=False, stop=last, perf_mode=DR,
                        )

                # Lag-1 pipeline: iteration j emits pair j+1's scores, pair
                # j's quantize chain, and pair j-1's readouts, so the
                # ACT->Pool->DVE chain has a two-pair window before PE needs
                # its result.
                pair_scores = pre_scores
                pending = None  # (j, e1, e2) awaiting readouts
                for j in range(NPAIR):
                    sa, sb = pair_scores
                    if j + 1 < NPAIR:
                        pair_scores = emit_scores(q0, nq, j + 1)
                    if j == 0 and pre_quant is not None:
                        e1, e2 = pre_quant
                    else:
                        e1, e2 = emit_quant(nq, j, sa, sb)
                    if pending is not None:
                        emit_reduce(*pending)
                    pending = (j, e1, e2)
                if qi + 1 < len(QBLKS):
                    # Pre-emit the next block's first-pair scores AND its
                    # quantize chain: PE chews the scores while the last
                    # pair drains, and crucially the next block's e2' lands
                    # on DVE ahead of this block's output muls.
                    nq0, nq1 = QBLKS[qi + 1]
                    pre_scores = emit_scores(nq0, nq1, 0)
                    pre_quant = emit_quant(nq1, 0, pre_scores[0], pre_scores[1])
                emit_reduce(*pending)

                for pair in range(2):
                    o2 = opool.tile([P, 2, nq], bf16, tag="out", name="out")
                    for jj in range(2):
                        c = 2 * pair + jj
                        nc.vector.tensor_mul(o2[:, jj, :], accs[c][:], recip_h[0][:])
                    nc.sync.dma_start(
                        out=out[:, 2 * pair : 2 * pair + 2, q0 : q0 + nq], in_=o2[:]
                    )

    nc.compile()
    return nc


def _get_program():
    global _PROGRAM
    if _PROGRAM is None:
        _PROGRAM = _build_program()
    return _PROGRAM


def _make_in_maps(mk, qk, mv):
    E4 = ml_dtypes.float8_e4m3
    mkf = np.ascontiguousarray(mk.reshape(B, CK, THW), dtype=np.float32)
    qkf = np.ascontiguousarray(qk.reshape(B, CK, NQ), dtype=np.float32)
    mvf = mv.reshape(B, CV, THW)
    onesd = np.ones((P, 2, P), dtype=E4)

    def pack(v):
        # [THWP, CV] -> [NPAIR, 2, 128, CV] -> [128, NPAIR, 2, CV] -> flat
        return np.ascontiguousarray(
            v.reshape(NPAIR, 2, P, CV).transpose(2, 0, 1, 3).reshape(P, NPAIR * PBLK)
        )

    in_maps = []
    for b in range(B):
        mkq_b = np.concatenate([mkf[b], mkf[b] * mkf[b]], axis=0)  # [128, THW]
        mvt = np.zeros((THWP, CV), dtype=np.float32)
        mvt[:THW] = mvf[b].T
        v1 = mvt.astype(E4)
        v2 = (mvt - v1.astype(np.float32)).astype(E4)
        v1p, v2p = pack(v1), pack(v2)
        for h in range(2):
            qkc_b = np.concatenate(
                [
                    qkf[b][:, h * QH : (h + 1) * QH],
                    np.full((CK, QH), -0.5, dtype=np.float32),
                ],
                axis=0,
            )
            in_maps.append(
                {
                    "mkq": mkq_b,
                    "qkc": np.ascontiguousarray(qkc_b),
                    "v1d": v1p,
                    "v2d": v2p,
                    "onesd": onesd,
                }
            )
    return in_maps


def kernel(mk, qk, mv, _trace=False, _results_out=None):
    from concourse import bass_utils

    nc = _get_program()
    in_maps = _make_in_maps(np.asarray(mk), np.asarray(qk), np.asarray(mv))
    res = bass_utils.run_bass_kernel_spmd(
        nc, in_maps, core_ids=list(range(8)), trace=_trace
    )
    if _results_out is not None:
        _results_out.append(res)

    full = np.empty((B, CV, NQ), dtype=np.float32)
    for b in range(B):
        for h in range(2):
            o = res.results[2 * b + h]["out"].astype(np.float32)  # [128, 4, QH]
            full[b][:, h * QH : (h + 1) * QH] = o.transpose(1, 0, 2).reshape(CV, QH)
    return full.reshape(B, CV, H, W)
